# Initial kernel scaffold
#
"""LocallyConnected2d (B=8, C_in=32, 48x48, C_out=32, 3x3, pad 1) on 8 trn2 cores.

Strategy: shard the spatial-location axis L = H*W across cores (6 image rows
each). Per location l the op is an (8x288)@(288x32) GEMM with location-unique
weights; weight streaming (85 MB total) dominates -> memory-bound.

Device mapping per core:
  - x halo slice lives in SBUF replicated 3x with kw column shifts, laid out
    [p=(kw*32+c), (row, col, b)], so the im2col patch for any location is a
    plain strided AP slice (no patch materialization).
  - Contraction (d=288) is split into 3 kh-rounds of K=96=(3 kw x 32 c),
    PSUM-accumulated. K=96 everywhere keeps one PE tiling mode (no drains);
    mixed-K designs either mode-switch per matmul or hit the "row tiles
    sharing a PSUM bank" hardware fault.
  - 4 consecutive locations are column-packed onto the PE with
    tile_position=(0, 32j): stationary = x-view [96, 8(b)] into column group
    j, moving = W slice [96, 32(o)], out = PSUM partitions 32j..32j+8. The
    four matmuls per (m, kh) execute concurrently on disjoint column groups.
  - Bias is added by one K=96 matmul per (group, j): a host-baked one-hot
    column picks the group's row out of a [96, 512] bias table (rows >=18
    zeroed on device), so the op shares the (128, 32) tiling mode.
  - W is host-permuted into per-(kh, LG-location) tiles that are fully
    contiguous in HBM with 9216-byte partition rows ([96, 9216B] DMAs
    measured ~193 GB/s vs ~94 GB/s naive); output is a [128, *] fp32 tile
    ((j,b) partitions x (group, m, o) free) stored with one fast DMA and
    unscrambled to NCHW on the host.
"""

import numpy as np

import concourse.bacc as bacc
import concourse.tile as tile
from concourse import mybir
from concourse.bass_utils import run_bass_kernel_spmd

B, C_IN, H, W = 8, 32, 48, 48
C_OUT = 32
N_CORES = 8
RP = H // N_CORES  # rows per core (6)
LP = RP * W  # locations per core (288)
NGRP = LP // 16  # 16-loc output groups per core (18)

DT16 = True  # fp16 operand path (halves weight traffic)
DT = mybir.dt.float16 if DT16 else mybir.dt.float32
NPDT = np.float16 if DT16 else np.float32
LG = 48  # locs per W tile (all 3 kh rounds per tile)
NT = LP // LG  # W tiles (6)
SF = 0  # placeholder
XF = (RP + 2) * W * B  # x3 free size (3072)
F32 = mybir.dt.float32

_nc = None


def _build():
    nc = bacc.Bacc(
        "TRN2", target_bir_lowering=False, debug=False, num_devices=N_CORES
    )
    SF = XF + NGRP * 32 + 512  # combined static tile free size
    stat = nc.dram_tensor("stat", [96, SF], DT, kind="ExternalInput")
    TILES = [(0, 16), (16, 32)] + [(48 * i, 48) for i in range(1, NT)]
    wds = [
        nc.dram_tensor(f"w{i}", [96, 3 * n * C_OUT], DT, kind="ExternalInput")
        for i, (_, n) in enumerate(TILES)
    ]
    out = nc.dram_tensor("out", [128, NGRP * 128], F32, kind="ExternalOutput")

    with tile.TileContext(nc) as tc:
        with (
            tc.tile_pool(name="xpool", bufs=1) as xpool,
            tc.tile_pool(name="wpool", bufs=4) as wpool,
            tc.tile_pool(name="opool", bufs=1) as opool,
            tc.tile_pool(name="pspool", bufs=8, space="PSUM") as pspool,
        ):
            stat_sb = xpool.tile([96, SF], DT, tag="stat")
            nc.gpsimd.dma_start(stat_sb[:, 0:XF], stat[:, 0:XF])
            nc.gpsimd.dma_start(stat_sb[:, XF:SF], stat[:, XF:SF])
            x3 = stat_sb[:, 0:XF]
            oneh_sb = stat_sb[:, XF : XF + NGRP * 32]
            bi_sb = stat_sb[:, XF + NGRP * 32 : SF]

            out_sb = opool.tile([128, NGRP * 128], F32)

            for t, (tl0, tn) in enumerate(TILES):
                wt = wpool.tile([96, 3 * 48 * C_OUT], DT, tag="wt")
                nc.gpsimd.dma_start(wt[0:96, 0 : 3 * tn * C_OUT], wds[t][:, :])
                for gl in range(tn // 16):
                    gi = tl0 // 16 + gl
                    rl, qg = divmod(gi, 3)
                    ps = pspool.tile([128, 512], F32)
                    for j in range(4):
                        nc.tensor.matmul(
                            ps[32 * j : 32 * j + 32, 0:128],
                            oneh_sb[0:96, gi * 32 : gi * 32 + 32],
                            bi_sb[0:96, j * 128 : (j + 1) * 128],
                            start=True,
                            stop=False,
                            skip_group_check=True,
                            tile_position=(0, 32 * j),
                        )
                    for m in range(4):
                        for kh in range(3):
                            for j in range(4):
                                q = qg * 16 + m * 4 + j
                                l = rl * W + q
                                ll = l - tl0
                                off = ((rl + kh) * W + q) * B
                                nc.tensor.matmul(
                                    ps[32 * j : 32 * j + B, m * 32 : (m + 1) * 32],
                                    x3[0:96, off : off + B],
                                    wt[0:96, (kh * tn + ll) * 32 : (kh * tn + ll + 1) * 32],
                                    start=False,
                                    stop=(m == 3 and kh == 2),
                                    skip_group_check=True,
                                    tile_position=(0, 32 * j),
                                )
                    nc.vector.tensor_copy(
                        out_sb[0:128, gi * 128 : (gi + 1) * 128], ps[0:128, 0:128]
                    )
            for c0 in range(0, NGRP, 6):
                nc.gpsimd.dma_start(
                    out[:, c0 * 128 : (c0 + 6) * 128],
                    out_sb[0:128, c0 * 128 : (c0 + 6) * 128],
                )
    nc.compile()
    return nc


def _shard(inputs):
    x = np.asarray(inputs["x"], np.float32)
    weight = np.asarray(inputs["weight"], np.float32)[0]
    bias = np.asarray(inputs["bias"], np.float32)[0]
    xp = np.pad(x, ((0, 0), (0, 0), (1, 1), (1, 1)))  # (b, c, 50, 50)
    bias_t = bias.reshape(C_OUT, H * W).T  # (L, C_OUT)
    wflat = weight.reshape(C_IN, 3, 3, H * W, C_OUT)  # (c, kh, kw, l, o)

    # one-hot group selector [96, NGRP*32] (cols m>=8 zero)
    oneh = np.zeros((96, NGRP * 32), NPDT)
    for gi in range(NGRP):
        oneh[gi, gi * 32 : gi * 32 + 8] = 1.0

    in_maps = []
    for k in range(N_CORES):
        r0 = RP * k
        l0 = LP * k

        x3h = np.empty((3, C_IN, RP + 2, W, B), np.float32)
        for kw in range(3):
            x3h[kw] = xp[:, :, r0 : r0 + RP + 2, kw : kw + W].transpose(1, 2, 3, 0)

        # W: per tile [(kw c), (kh, lg, o)]
        wk = wflat[:, :, :, l0 : l0 + LP, :]  # (c, kh, kw, LP, o)
        wall = wk.transpose(2, 0, 1, 3, 4).reshape(96, 3, LP, C_OUT)
        tiles = [(0, 16), (16, 32)] + [(48 * i, 48) for i in range(1, LP // 48)]
        wtiles = {
            f"w{i}": np.ascontiguousarray(
                wall[:, :, t0 : t0 + n, :].reshape(96, 3 * n * C_OUT)
            ).astype(NPDT)
            for i, (t0, n) in enumerate(tiles)
        }

        # bias rows per group: (j, m, o)
        bk = bias_t[l0 : l0 + LP, :].reshape(NGRP, 4, 4, C_OUT)  # (gi, m, j, o)
        bi = bk.transpose(0, 2, 1, 3).reshape(NGRP, 512)  # (gi, (j, m, o))

        stat = np.zeros((96, XF + NGRP * 32 + 512), NPDT)
        stat[:, 0:XF] = x3h.reshape(96, XF).astype(NPDT)
        stat[:, XF : XF + NGRP * 32] = oneh
        stat[0:NGRP, XF + NGRP * 32 :] = bi.astype(NPDT)
        m = {"stat": stat}
        m.update(wtiles)
        in_maps.append(m)
    return in_maps


def _get_nc():
    global _nc
    if _nc is None:
        _nc = _build()
    return _nc


def _gather(results):
    # out rows 32j+b (b<8) hold y[b, o, r, q] at col gi*128 + m*32 + o,
    # with r = gi//3, q = (gi%3)*16 + m*4 + j
    y = np.empty((B, C_OUT, H, W), np.float32)
    for k in range(N_CORES):
        arr = results[k]["out"].reshape(4, 32, NGRP, 4, C_OUT)  # (j, b*, gi, m, o)
        arr = arr[:, 0:B]  # (j, b, gi, m, o)
        arr = arr.transpose(1, 4, 2, 3, 0)  # (b, o, gi, m, j)
        arr = arr.reshape(B, C_OUT, RP, 3, 4, 4)  # (b, o, r, qg, m, j)
        y[:, :, RP * k : RP * (k + 1), :] = arr.reshape(B, C_OUT, RP, W)
    return y


def kernel(**inputs):
    nc = _get_nc()
    res = run_bass_kernel_spmd(nc, _shard(inputs), list(range(N_CORES)))
    return _gather(res.results)



# revision 1
# speedup vs baseline: 3.8227x; 3.8227x over previous
"""LocallyConnected2d (B=8, C_in=32, 48x48, C_out=32, 3x3, pad 1) on 8 trn2 cores.

Strategy: shard the spatial-location axis L = H*W across cores (6 image rows
each). Per location l the op is an (8x288)@(288x32) GEMM with location-unique
weights; weight streaming (85 MB total) dominates -> memory-bound.

Device mapping per core:
  - x halo slice lives in SBUF replicated 3x with kw column shifts, laid out
    [p=(kw*32+c), (row, col, b)], so the im2col patch for any location is a
    plain strided AP slice (no patch materialization).
  - Contraction (d=288) is split into 3 kh-rounds of K=96=(3 kw x 32 c),
    PSUM-accumulated. K=96 everywhere keeps one PE tiling mode (no drains);
    mixed-K designs either mode-switch per matmul or hit the "row tiles
    sharing a PSUM bank" hardware fault.
  - 4 consecutive locations are column-packed onto the PE with
    tile_position=(0, 32j): stationary = x-view [96, 8(b)] into column group
    j, moving = W slice [96, 32(o)], out = PSUM partitions 32j..32j+8. The
    four matmuls per (m, kh) execute concurrently on disjoint column groups.
  - Bias is added by one K=96 matmul per (group, j): a host-baked one-hot
    column picks the group's row out of a [96, 512] bias table (rows >=18
    zeroed on device), so the op shares the (128, 32) tiling mode.
  - W is host-permuted into per-(kh, LG-location) tiles that are fully
    contiguous in HBM with 9216-byte partition rows ([96, 9216B] DMAs
    measured ~193 GB/s vs ~94 GB/s naive); output is a [128, *] fp32 tile
    ((j,b) partitions x (group, m, o) free) stored with one fast DMA and
    unscrambled to NCHW on the host.
"""

import numpy as np

import concourse.bacc as bacc
import concourse.tile as tile
from concourse import mybir
from concourse.bass_utils import run_bass_kernel_spmd

B, C_IN, H, W = 8, 32, 48, 48
C_OUT = 32
N_CORES = 8
RP = H // N_CORES  # rows per core (6)
LP = RP * W  # locations per core (288)
NGRP = LP // 16  # 16-loc output groups per core (18)

DT16 = True  # fp16 operand path (halves weight traffic)
DT = mybir.dt.float16 if DT16 else mybir.dt.float32
NPDT = np.float16 if DT16 else np.float32
LG = 48  # locs per W tile (all 3 kh rounds per tile)
NT = LP // LG  # W tiles (6)
SF = 0  # placeholder
XF = (RP + 2) * W * B  # x3 free size (3072)
F32 = mybir.dt.float32

_nc = None


def _build():
    nc = bacc.Bacc(
        "TRN2", target_bir_lowering=False, debug=False, num_devices=N_CORES
    )
    SF = XF + NGRP * 32 + 512  # combined static tile free size
    stat = nc.dram_tensor("stat", [96, SF], DT, kind="ExternalInput")
    TILES = [(0, 16), (16, 32)] + [(48 * i, 48) for i in range(1, NT)]
    wds = [
        nc.dram_tensor(f"w{i}", [96, 3 * n * C_OUT], DT, kind="ExternalInput")
        for i, (_, n) in enumerate(TILES)
    ]
    out = nc.dram_tensor("out", [128, NGRP * 128], F32, kind="ExternalOutput")

    with tile.TileContext(nc) as tc:
        with (
            tc.tile_pool(name="xpool", bufs=1) as xpool,
            tc.tile_pool(name="wpool", bufs=4) as wpool,
            tc.tile_pool(name="opool", bufs=1) as opool,
            tc.tile_pool(name="pspool", bufs=8, space="PSUM") as pspool,
        ):
            stat_sb = xpool.tile([96, SF], DT, tag="stat")
            nc.gpsimd.dma_start(stat_sb[:, 0:XF], stat[:, 0:XF])
            nc.gpsimd.dma_start(stat_sb[:, XF:SF], stat[:, XF:SF])
            x3 = stat_sb[:, 0:XF]
            oneh_sb = stat_sb[:, XF : XF + NGRP * 32]
            bi_sb = stat_sb[:, XF + NGRP * 32 : SF]

            out_sb = opool.tile([128, NGRP * 128], F32)

            for t, (tl0, tn) in enumerate(TILES):
                wt = wpool.tile([96, 3 * 48 * C_OUT], DT, tag="wt")
                nc.gpsimd.dma_start(wt[0:96, 0 : 3 * tn * C_OUT], wds[t][:, :])
                for gl in range(tn // 16):
                    gi = tl0 // 16 + gl
                    rl, qg = divmod(gi, 3)
                    ps = pspool.tile([128, 512], F32)
                    for j in range(4):
                        nc.tensor.matmul(
                            ps[32 * j : 32 * j + 32, 0:128],
                            oneh_sb[0:96, gi * 32 : gi * 32 + 32],
                            bi_sb[0:96, j * 128 : (j + 1) * 128],
                            start=True,
                            stop=False,
                            skip_group_check=True,
                            tile_position=(0, 32 * j),
                        )
                    for m in range(4):
                        for kh in range(3):
                            for j in range(4):
                                q = qg * 16 + m * 4 + j
                                l = rl * W + q
                                ll = l - tl0
                                off = ((rl + kh) * W + q) * B
                                nc.tensor.matmul(
                                    ps[32 * j : 32 * j + B, m * 32 : (m + 1) * 32],
                                    x3[0:96, off : off + B],
                                    wt[0:96, (kh * tn + ll) * 32 : (kh * tn + ll + 1) * 32],
                                    start=False,
                                    stop=(m == 3 and kh == 2),
                                    skip_group_check=True,
                                    tile_position=(0, 32 * j),
                                )
                    nc.vector.tensor_copy(
                        out_sb[0:128, gi * 128 : (gi + 1) * 128], ps[0:128, 0:128]
                    )
            for c0 in range(0, NGRP, 6):
                nc.gpsimd.dma_start(
                    out[:, c0 * 128 : (c0 + 6) * 128],
                    out_sb[0:128, c0 * 128 : (c0 + 6) * 128],
                )
    nc.compile()
    return nc


def _shard(inputs):
    x = np.asarray(inputs["x"], np.float32)
    weight = np.asarray(inputs["weight"], np.float32)[0]
    bias = np.asarray(inputs["bias"], np.float32)[0]
    xp = np.pad(x, ((0, 0), (0, 0), (1, 1), (1, 1)))  # (b, c, 50, 50)
    bias_t = bias.reshape(C_OUT, H * W).T  # (L, C_OUT)
    wflat = weight.reshape(C_IN, 3, 3, H * W, C_OUT)  # (c, kh, kw, l, o)

    # one-hot group selector [96, NGRP*32] (cols m>=8 zero)
    oneh = np.zeros((96, NGRP * 32), NPDT)
    for gi in range(NGRP):
        oneh[gi, gi * 32 : gi * 32 + 8] = 1.0

    in_maps = []
    for k in range(N_CORES):
        r0 = RP * k
        l0 = LP * k

        x3h = np.empty((3, C_IN, RP + 2, W, B), np.float32)
        for kw in range(3):
            x3h[kw] = xp[:, :, r0 : r0 + RP + 2, kw : kw + W].transpose(1, 2, 3, 0)

        # W: per tile [(kw c), (kh, lg, o)]
        wk = wflat[:, :, :, l0 : l0 + LP, :]  # (c, kh, kw, LP, o)
        wall = wk.transpose(2, 0, 1, 3, 4).reshape(96, 3, LP, C_OUT)
        tiles = [(0, 16), (16, 32)] + [(48 * i, 48) for i in range(1, LP // 48)]
        wtiles = {
            f"w{i}": np.ascontiguousarray(
                wall[:, :, t0 : t0 + n, :].reshape(96, 3 * n * C_OUT)
            ).astype(NPDT)
            for i, (t0, n) in enumerate(tiles)
        }

        # bias rows per group: (j, m, o)
        bk = bias_t[l0 : l0 + LP, :].reshape(NGRP, 4, 4, C_OUT)  # (gi, m, j, o)
        bi = bk.transpose(0, 2, 1, 3).reshape(NGRP, 512)  # (gi, (j, m, o))

        stat = np.zeros((96, XF + NGRP * 32 + 512), NPDT)
        stat[:, 0:XF] = x3h.reshape(96, XF).astype(NPDT)
        stat[:, XF : XF + NGRP * 32] = oneh
        stat[0:NGRP, XF + NGRP * 32 :] = bi.astype(NPDT)
        m = {"stat": stat}
        m.update(wtiles)
        in_maps.append(m)
    return in_maps


def _get_nc():
    global _nc
    if _nc is None:
        _nc = _build()
    return _nc


def _gather(results):
    # out rows 32j+b (b<8) hold y[b, o, r, q] at col gi*128 + m*32 + o,
    # with r = gi//3, q = (gi%3)*16 + m*4 + j
    y = np.empty((B, C_OUT, H, W), np.float32)
    for k in range(N_CORES):
        arr = results[k]["out"].reshape(4, 32, NGRP, 4, C_OUT)  # (j, b*, gi, m, o)
        arr = arr[:, 0:B]  # (j, b, gi, m, o)
        arr = arr.transpose(1, 4, 2, 3, 0)  # (b, o, gi, m, j)
        arr = arr.reshape(B, C_OUT, RP, 3, 4, 4)  # (b, o, r, qg, m, j)
        y[:, :, RP * k : RP * (k + 1), :] = arr.reshape(B, C_OUT, RP, W)
    return y


def kernel(**inputs):
    nc = _get_nc()
    res = run_bass_kernel_spmd(nc, _shard(inputs), list(range(N_CORES)))
    return _gather(res.results)

